# revision 1
# baseline (speedup 1.0000x reference)
"""Trainium2 Bass kernel for a cross-attention block.

Reference computation (per batch b of 2):
  qc   = conv3x3(q)                      # [256, 64, 64], SAME padding
  qn   = rmsnorm(qc, over channel) * g_q
  kn   = rmsnorm(k,  over channel) * g_k
  qp   = qn @ wq.T + bq                  # [4096, 256] -> 8 heads x 32
  kp   = kn @ wk.T + bk                  # [1024, 256]
  s    = qp . kp / sqrt(32) per head, masked to a local window
  attn = mean_h softmax_k(s)             # [4096, 1024]
  out  = attn @ v_flat                   # [4096, 256] -> [256, 64, 64]

Sharding: 8 cores = (batch 2) x (16-row query stripes 4). Each core computes
its stripe's conv (with halo rows sent from the host), full k/v projections
for the 14 key rows its queries can see, and windowed masked attention.

Query tiling inside a core: 8 tiles of 128 queries = 16 rows x 8 cols.
Each tile sees a 14x10 key window (140 keys), handled densely with a
host-precomputed 0/1 mask; kpT / v are laid out kj-major so every window is
a contiguous column/row range.
"""

from contextlib import ExitStack

import numpy as np

import concourse.bacc as bacc
import concourse.bass as bass
import concourse.tile as tile
from concourse import mybir
from concourse.bass_utils import run_bass_kernel_spmd

F32 = mybir.dt.float32
BF16 = mybir.dt.bfloat16
AF = mybir.ActivationFunctionType
ALU = mybir.AluOpType

B, C, H, W = 2, 256, 64, 64
HK, WK = 32, 32
NH, HD = 8, 32
EPS = 1e-6
SCALE = 1.0 / np.sqrt(HD)

NCORES = 8
RSTRIPE = 16            # query rows per core
KI = 14                 # key rows per core window
KJ = 10                 # key cols per q-tile window
NW = KJ * KI            # 140 keys per q-tile window
NT = 8                  # q-tiles per core (16y x 8x each)
KJ0 = [max(0, min(4 * s - 3, WK - KJ)) for s in range(NT)]
KI0 = [max(0, min(8 * r - 3, HK - KI)) for r in range(4)]

# dtype knobs (phase-2 tuning)
F32R = mybir.dt.float32r
CONV_DT = BF16          # conv operands bf16 (full-rate matmul, psum accumulates f32)
SCORE_DT = F32          # dtype of qpT/kpT for score matmuls
ATTN_DT = F32           # dtype of attn/v for the feat matmul
EXP_DT = F32            # dtype of exp'd scores


import os

DBG_STAGE = int(os.environ.get("DBG_STAGE", "9"))  # 9 = full kernel
DBG_SUB = int(os.environ.get("DBG_SUB", "4"))


def build_nc():
    nc = bacc.Bacc()
    qpad_d = nc.declare_dram_parameter("qpad", [2, 128, 18, 66], CONV_DT, isOutput=False)
    wt_d = nc.declare_dram_parameter("wt", [2, 128, 9, 256], CONV_DT, isOutput=False)
    wqt_d = nc.declare_dram_parameter("wqt", [2, 128, 256], F32, isOutput=False)
    bq_d = nc.declare_dram_parameter("bqv", [2, 128, 1], F32, isOutput=False)
    wkt_d = nc.declare_dram_parameter("wkt", [2, 128, 256], F32, isOutput=False)
    bk_d = nc.declare_dram_parameter("bkv", [2, 128, 1], F32, isOutput=False)
    kin_d = nc.declare_dram_parameter("kin", [2, 128, 448], F32, isOutput=False)
    vin_d = nc.declare_dram_parameter("vin", [448, 256], ATTN_DT, isOutput=False)
    msk_d = nc.declare_dram_parameter("msk", [NT, 128, NW], F32, isOutput=False)
    id_d = nc.declare_dram_parameter("ident", [128, 128], ATTN_DT, isOutput=False)
    out_d = nc.declare_dram_parameter("out", [256, RSTRIPE, 64], F32, isOutput=True)

    with tile.TileContext(nc) as tc, ExitStack() as ctx:
        singles = ctx.enter_context(tc.tile_pool(name="singles", bufs=1))
        work = ctx.enter_context(tc.tile_pool(name="work", bufs=1))

        # ---- load all persistent inputs ----
        qpad_t = []
        wt_t = []
        wqt_t = []
        wkt_t = []
        bq_t = []
        bk_t = []
        kin_t = []
        for ct in range(2):
            qp_ = singles.tile([128, 18, 66], CONV_DT, name=f"qpad{ct}")
            nc.sync.dma_start(qp_[:], qpad_d[ct])
            qpad_t.append(qp_)
            wt_ = singles.tile([128, 9, 256], CONV_DT, name=f"wt{ct}")
            nc.sync.dma_start(wt_[:], wt_d[ct])
            wt_t.append(wt_)
            wq_ = singles.tile([128, 256], F32, name=f"wqt{ct}")
            nc.sync.dma_start(wq_[:], wqt_d[ct])
            wqt_t.append(wq_)
            wk_ = singles.tile([128, 256], F32, name=f"wkt{ct}")
            nc.sync.dma_start(wk_[:], wkt_d[ct])
            wkt_t.append(wk_)
            bq_ = singles.tile([128, 1], F32, name=f"bq{ct}")
            nc.sync.dma_start(bq_[:], bq_d[ct])
            bq_t.append(bq_)
            bk_ = singles.tile([128, 1], F32, name=f"bk{ct}")
            nc.sync.dma_start(bk_[:], bk_d[ct])
            bk_t.append(bk_)
            ki_ = singles.tile([128, 448], F32, name=f"kin{ct}")
            nc.sync.dma_start(ki_[:], kin_d[ct])
            kin_t.append(ki_)
        msk_t = singles.tile([128, NT, NW], F32)
        nc.sync.dma_start(msk_t[:], msk_d.ap().rearrange("s q w -> q s w"))
        ident_t = singles.tile([128, 128], ATTN_DT)
        nc.sync.dma_start(ident_t[:], id_d[:])
        ones_col = singles.tile([128, 1], F32)
        nc.vector.memset(ones_col[:], 1.0)
        ones_row = singles.tile([1, 128], F32)
        nc.vector.memset(ones_row[:], 1.0)
        eps_t = singles.tile([1, 1], F32)
        nc.vector.memset(eps_t[:], EPS)

        qcT = [work.tile([128, 1024], F32, name=f"qcT{i}") for i in range(2)]
        sq = [work.tile([128, 1024], F32, name=f"sq{i}") for i in range(2)]
        qn = [work.tile([128, 1024], F32, name=f"qn{i}") for i in range(2)]
        # tile-major [co, s-tile, 128q] so score matmuls get a contiguous lhsT
        qpT = [work.tile([128, NT, 128], SCORE_DT, name=f"qpT{i}") for i in range(2)]
        sqk = [work.tile([128, 448], F32, name=f"sqk{i}") for i in range(2)]
        kn = [work.tile([128, 448], F32, name=f"kn{i}") for i in range(2)]
        kpT = [work.tile([128, 448], SCORE_DT, name=f"kpT{i}") for i in range(2)]

        # ---- conv 3x3 (as 18 accumulated shifted matmuls per co-tile) ----
        with tc.tile_pool(name="ps_conv", bufs=1, space="PSUM") as pscv:
            for co_t in range(2):
                ps = [
                    pscv.tile([128, 512], F32, name=f"cv{co_t}_{n2}", tag=f"cv{n2}", bufs=2)
                    for n2 in range(2)
                ]
                for ci in range(2):
                    for tap in range(9):
                        dy, dx = divmod(tap, 3)
                        lhsT = wt_t[ci][:, tap, 128 * co_t : 128 * (co_t + 1)]
                        for n2 in range(2):
                            rhs = qpad_t[ci][:, dy + 8 * n2 : dy + 8 * n2 + 8, dx : dx + 64]
                            nc.tensor.matmul(
                                ps[n2][:],
                                lhsT,
                                rhs,
                                start=(ci == 0 and tap == 0),
                                stop=(ci == 1 and tap == 8),
                            )
                for n2 in range(2):
                    sl = slice(512 * n2, 512 * (n2 + 1))
                    nc.vector.tensor_copy(qcT[co_t][:, sl], ps[n2][:])
                    nc.scalar.square(sq[co_t][:, sl], ps[n2][:])

        if DBG_STAGE == 1:
            # bypass rmsnorm: qn = qcT, kn = kin (tests conv/proj matmuls only)
            for ct in range(2):
                nc.vector.tensor_copy(qn[ct][:], qcT[ct][:])
                nc.vector.tensor_copy(kn[ct][:], kin_t[ct][:])

        # ---- rmsnorm of conv output (reduce over channel = partition dim) ----
        with tc.tile_pool(name="ps_norm", bufs=1, space="PSUM") as psn:
          if DBG_STAGE != 1:
              rinv_q = work.tile([1, 1024], F32)
              for n2 in range(2):
                  sl = slice(512 * n2, 512 * (n2 + 1))
                  ms = psn.tile([1, 512], F32, tag="ms", bufs=2)
                  for ct in range(2):
                      nc.tensor.matmul(
                          ms[:], ones_col[:], sq[ct][:, sl], start=(ct == 0), stop=(ct == 1)
                      )
                  tmp = work.tile([1, 512], F32, tag="rtmp", bufs=2)
                  nc.scalar.activation(tmp[:], ms[:], AF.Sqrt, bias=eps_t[:], scale=1.0 / C)
                  nc.vector.reciprocal(rinv_q[:, sl], tmp[:])
              for n2 in range(2):
                  sl = slice(512 * n2, 512 * (n2 + 1))
                  rb = psn.tile([128, 512], F32, tag="rb", bufs=2)
                  nc.tensor.matmul(rb[:], ones_row[:], rinv_q[:, sl], start=True, stop=True)
                  for ct in range(2):
                      nc.vector.tensor_mul(qn[ct][:, sl], qcT[ct][:, sl], rb[:])

              # k-side rmsnorm (448 columns)
              for ct in range(2):
                  nc.scalar.square(sqk[ct][:], kin_t[ct][:])
              msk_ = psn.tile([1, 448], F32, tag="msk", bufs=1)
              for ct in range(2):
                  nc.tensor.matmul(
                      msk_[:], ones_col[:], sqk[ct][:], start=(ct == 0), stop=(ct == 1)
                  )
              tmpk = work.tile([1, 448], F32)
              nc.scalar.activation(tmpk[:], msk_[:], AF.Sqrt, bias=eps_t[:], scale=1.0 / C)
              rinv_k = work.tile([1, 448], F32)
              nc.vector.reciprocal(rinv_k[:], tmpk[:])
              rbk = psn.tile([128, 448], F32, tag="rbk", bufs=1)
              nc.tensor.matmul(rbk[:], ones_row[:], rinv_k[:], start=True, stop=True)
              for ct in range(2):
                  nc.vector.tensor_mul(kn[ct][:], kin_t[ct][:], rbk[:])

        # ---- q / k projections (into transposed [co, token] layout) ----
        with tc.tile_pool(name="ps_proj", bufs=1, space="PSUM") as psp:
            for co_t in range(2):
                for n2 in range(2):
                    pq = psp.tile([128, 4, 128], F32, tag="pq", bufs=2)
                    for si in range(4):
                        s = 4 * n2 + si
                        for ct in range(2):
                            # moving operand: 16 rows x 8 cols of this q-tile
                            rhs = qn[ct][:].rearrange("p (y x) -> p y x", x=64)[
                                :, :, 8 * s : 8 * (s + 1)
                            ]
                            nc.tensor.matmul(
                                pq[:, si, :],
                                wqt_t[ct][:, 128 * co_t : 128 * (co_t + 1)],
                                rhs,
                                start=(ct == 0),
                                stop=(ct == 1),
                            )
                    qpT_flat = qpT[co_t][:].rearrange("p s q -> p (s q)")
                    nc.vector.tensor_scalar_add(
                        qpT_flat[:, 512 * n2 : 512 * (n2 + 1)],
                        pq[:].rearrange("p s q -> p (s q)"),
                        bq_t[co_t][:],
                    )
                pk = psp.tile([128, 448], F32, tag="pk", bufs=2)
                for ct in range(2):
                    nc.tensor.matmul(
                        pk[:],
                        wkt_t[ct][:, 128 * co_t : 128 * (co_t + 1)],
                        kn[ct][:],
                        start=(ct == 0),
                        stop=(ct == 1),
                    )
                nc.vector.tensor_scalar_add(kpT[co_t][:], pk[:], bk_t[co_t][:])

        # ---- windowed masked attention, one 128-query tile at a time ----
        if DBG_STAGE < 2:
            # debug: dump qpT instead of attention output
            dbg = ctx.enter_context(tc.tile_pool(name="dbg", bufs=2))
            for co_t in range(2):
                for s in range(NT):
                    fo = dbg.tile([128, 128], F32, tag="fo", bufs=2)
                    nc.vector.tensor_copy(fo[:], qpT[co_t][:, s, :])
                    nc.sync.dma_start(
                        out_d[128 * co_t : 128 * (co_t + 1), :, 8 * s : 8 * (s + 1)],
                        fo[:].rearrange("d (y x) -> d y x", x=8),
                    )

        att = ctx.enter_context(tc.tile_pool(name="att", bufs=2))
        with tc.tile_pool(name="ps_att", bufs=1, space="PSUM") as psa:
            for s in range(NT if DBG_STAGE >= 2 else 0):
                kj0 = KJ0[s]
                vwin = [
                    att.tile([70, 256], ATTN_DT, name=f"vw{c}", tag=f"vw{c}", bufs=2)
                    for c in range(2)
                ]
                for c in range(2):
                    nc.sync.dma_start(
                        vwin[c][:], vin_d[14 * kj0 + 70 * c : 14 * kj0 + 70 * (c + 1), :]
                    )
                e_t = att.tile([128, NH, NW], EXP_DT, tag="e", bufs=2)
                em_t = att.tile([128, NH, NW], EXP_DT, tag="em", bufs=2)
                sums = att.tile([128, NH], F32, tag="sums", bufs=2)
                rs = att.tile([128, NH], F32, tag="rs", bufs=2)
                for h in range(NH):
                    ht, hr = divmod(h, 4)
                    sc = psa.tile([128, NW], F32, tag=f"sc{h % 4}", bufs=1)
                    nc.tensor.matmul(
                        sc[:],
                        qpT[ht][32 * hr : 32 * hr + 32, s, :],
                        kpT[ht][32 * hr : 32 * hr + 32, 14 * kj0 : 14 * kj0 + NW],
                        start=True,
                        stop=True,
                        tile_position=(32 * hr, 0),
                    )
                    nc.scalar.activation(e_t[:, h, :], sc[:], AF.Exp)
                    if DBG_STAGE < 3 or DBG_SUB < 1:
                        continue
                    nc.vector.tensor_mul(em_t[:, h, :], e_t[:, h, :], msk_t[:, s, :])
                # one reduce over all heads: [128, 8, 140] -> [128, 8]
                if DBG_STAGE >= 3 and DBG_SUB >= 1:
                    nc.vector.reduce_sum(
                        out=sums[:], in_=em_t[:], axis=mybir.AxisListType.X
                    )
                if DBG_STAGE < 3 or DBG_SUB < 2:
                    continue
                nc.vector.reciprocal(rs[:], sums[:])
                attn = att.tile([128, NW], ATTN_DT, tag="attn0", bufs=2)
                nc.vector.tensor_scalar_mul(attn[:], em_t[:, 0, :], rs[:, 0:1])
                for h in range(1, NH if DBG_SUB >= 3 else 0):
                    attn2 = att.tile([128, NW], ATTN_DT, tag=f"attn{h}", bufs=2)
                    nc.vector.scalar_tensor_tensor(
                        out=attn2[:],
                        in0=em_t[:, h, :],
                        scalar=rs[:, h : h + 1],
                        in1=attn[:],
                        op0=ALU.mult,
                        op1=ALU.add,
                    )
                    attn = attn2
                if DBG_STAGE < 4:
                    continue
                # transpose attn -> [140, 128] in two 70-column chunks
                attnT = []
                for c in range(2):
                    tp = psa.tile([70, 128], ATTN_DT, tag=f"tp{c}", bufs=1)
                    nc.tensor.transpose(tp[:], attn[:, 70 * c : 70 * (c + 1)], ident_t[:])
                    atT = att.tile([70, 128], ATTN_DT, tag=f"atT{c}", bufs=2)
                    nc.vector.tensor_copy(atT[:], tp[:])
                    attnT.append(atT)
                if DBG_STAGE < 5:
                    continue
                for co_t in range(2):
                    ft = psa.tile([128, 128], F32, tag=f"ft{co_t}", bufs=1)
                    for c in range(2):
                        nc.tensor.matmul(
                            ft[:],
                            vwin[c][:, 128 * co_t : 128 * (co_t + 1)],
                            attnT[c][:],
                            start=(c == 0),
                            stop=(c == 1),
                        )
                    fo = att.tile([128, 128], F32, tag=f"fo{co_t}", bufs=2)
                    nc.vector.tensor_copy(fo[:], ft[:])
                    nc.sync.dma_start(
                        out_d[128 * co_t : 128 * (co_t + 1), :, 8 * s : 8 * (s + 1)],
                        fo[:].rearrange("d (y x) -> d y x", x=8),
                    )
    nc.compile()
    return nc


def _host_prep(q, k, v, conv_w, g_q, g_k, wq, bq, wk, bk):
    f = np.float32
    q = np.ascontiguousarray(q, dtype=f)
    k = np.ascontiguousarray(k, dtype=f)
    v = np.ascontiguousarray(v, dtype=f)
    wt = (
        np.ascontiguousarray(conv_w, dtype=f)
        .transpose(2, 3, 1, 0)
        .reshape(9, 2, 128, 256)
        .transpose(1, 2, 0, 3)
    )
    wt = np.ascontiguousarray(wt, dtype=mybir.dt.np(CONV_DT))
    wqt = np.ascontiguousarray(
        (wq.T * g_q[:, None] * SCALE).reshape(2, 128, 256), dtype=f
    )
    bqv = np.ascontiguousarray((bq * SCALE).reshape(2, 128, 1), dtype=f)
    wkt = np.ascontiguousarray((wk.T * g_k[:, None]).reshape(2, 128, 256), dtype=f)
    bkv = np.ascontiguousarray(bk.reshape(2, 128, 1), dtype=f)
    ident = np.eye(128, dtype=f)

    # masks per stripe r: [NT, 128, NW] with q = yl*8+xl, w = kjl*14 + kil
    masks = []
    for r in range(4):
        ki = KI0[r] + np.arange(KI, dtype=f)
        m_r = np.empty((NT, 128, NW), dtype=f)
        y = 16 * r + np.arange(RSTRIPE, dtype=f)
        ci = (y + 0.5) * 0.5 - 0.5
        oki = np.abs(ci[:, None] - ki[None, :]) <= 3.0  # [16, 14]
        for s in range(NT):
            kj = KJ0[s] + np.arange(KJ, dtype=f)
            x = 8 * s + np.arange(8, dtype=f)
            cj = (x + 0.5) * 0.5 - 0.5
            okj = np.abs(cj[:, None] - kj[None, :]) <= 3.0  # [8, 10]
            m = (
                oki[:, None, None, :] & okj[None, :, :, None]
            )  # [yl, xl, kjl, kil]
            m_r[s] = m.reshape(128, NW).astype(f)
        masks.append(m_r)

    in_maps = []
    for core in range(NCORES):
        b, r = divmod(core, 4)
        qpad = np.zeros((256, 18, 66), dtype=f)
        lo = max(0, 16 * r - 1)
        hi = min(64, 16 * r + 17)
        qpad[:, lo - (16 * r - 1) : hi - (16 * r - 1), 1:65] = q[b, :, lo:hi, :]
        ki0 = KI0[r]
        ksl = k[b][:, ki0 : ki0 + KI, :]  # [256, 14, 32]
        kin = np.ascontiguousarray(ksl.transpose(0, 2, 1).reshape(2, 128, 448), dtype=f)
        # 1/NH folds the mean-over-heads into the value matmul
        vin = np.ascontiguousarray(
            v[b][:, ki0 : ki0 + KI, :].transpose(2, 1, 0).reshape(448, 256) / NH,
            dtype=mybir.dt.np(ATTN_DT),
        )
        in_maps.append(
            {
                "qpad": qpad.reshape(2, 128, 18, 66).astype(mybir.dt.np(CONV_DT)),
                "wt": wt,
                "wqt": wqt,
                "bqv": bqv,
                "wkt": wkt,
                "bkv": bkv,
                "kin": kin,
                "vin": vin,
                "msk": masks[r],
                "ident": ident,
            }
        )
    return in_maps


_NC = None


def get_nc():
    global _NC
    if _NC is None:
        _NC = build_nc()
    return _NC


def kernel(q, k, v, conv_w, g_q, g_k, wq, bq, wk, bk):
    in_maps = _host_prep(q, k, v, conv_w, g_q, g_k, wq, bq, wk, bk)
    nc = get_nc()
    res = run_bass_kernel_spmd(nc, in_maps, list(range(NCORES)))
    out = np.empty((B, C, H, W), dtype=np.float32)
    for core in range(NCORES):
        b, r = divmod(core, 4)
        out[b, :, 16 * r : 16 * r + RSTRIPE, :] = res.results[core]["out"]
    return out

